# revision 30
# baseline (speedup 1.0000x reference)
"""MoE gate routing kernel (DeepSeek-V2-style group-limited top-k) for 8x TRN2 NeuronCores.

Problem: nn_MoEGate_13907104105110
  hidden_states [32768, 5120] fp32, gate weight [160, 5120] fp32
  logits = x @ W.T ; scores = softmax(logits)
  group-limited greedy top-k: 8 groups of 20 experts, keep top-3 groups by
  group max score, then top-6 scores of the kept groups, scaled by 16.0.
  Output: [32768, 6] fp32 (top-6 weights, descending).

Sharding: data-parallel over tokens; 4096 tokens per core, W replicated.

Default mode bf16x3p: x is hi/lo bf16-split AND transposed on host (numpy),
so the device kernel is a pure stream: DMA xT tile -> 3 bf16 matmuls per
128-hidden chunk (hi@Whi + hi@Wlo + lo@Whi, fp32-grade accuracy) -> softmax
via reduce_max(negate) + ACT Exp(bias=-max, accum_out=sum) -> group-limited
top-k on DVE.  No on-chip transpose, no PSUM->SBUF staging copies.
W is loaded as 5 per-chunk-group tiles so the first matmuls gate on 0.66MB,
not the whole 3.3MB; tile 0 splits its DMA hi/lo and runs hi-phase matmuls
first so PE starts after 1.3MB.  Top-k selection runs on raw exp-scores
(selection is scale-invariant); the softmax normalization + x16 scale is
applied only to the final [128, 8] tile.  Device time per core (cost-model
sim): ~281.3 us = 32 x 8.04 us/tile steady (the exact 480-cycle/chunk PE
stream floor) + ~7 us ramp/routing + ~17 us mandatory kernel-end drain;
DMA floor ~253 us.

Other modes kept for experiments:
  fp32:    on-chip PE transpose, 1 fp32 matmul/chunk (4 cyc/row)
  fp32r:   on-chip transpose, fp32r tf32-like (FAILS top-k accuracy: ~1e-4
           logit noise flips group/top-k selections for rare near-tie tokens)
  fp32rp:  host-transposed fp32r (same accuracy failure)
  bf16x3:  on-chip transpose + hi/lo split on device (prior baseline)

Routing (all modes), per 128-token tile, logits [128, E] in PSUM:
  softmax -> scores*16 -> group max (reduce over [128, 8, 20]) -> top-8
  (vector.max) -> 3rd value as group threshold -> is_ge mask -> mask scores
  -> top-8 of masked -> first 6 out.
"""

import sys

if "/opt/trn_rl_repo" not in sys.path:
    sys.path.insert(0, "/opt/trn_rl_repo")

from contextlib import ExitStack

import ml_dtypes
import numpy as np

import concourse.bass as bass
import concourse.mybir as mybir
from concourse import bacc
from concourse import tile
from concourse.bass_utils import run_bass_kernel_spmd
from concourse.masks import make_identity

TOKENS = 32768
HIDDEN = 5120
NEXP = 160
EPAD = 256  # fp32r needs moving free dim >= 256 for full rate
TOPK = 6
NGROUP = 8
EPG = NEXP // NGROUP  # 20 experts per group
TOPK_GROUP = 3
SCALE = 16.0
NCORES = 8
TPC = TOKENS // NCORES  # 4096 tokens per core
PT = 128  # tokens per tile
KC = HIDDEN // 128  # 40 contraction chunks

F32 = mybir.dt.float32
F32R = mybir.dt.float32r
BF16 = mybir.dt.bfloat16

MM_MODE = "bf16x3p"


def build_nc(tokens_per_core: int = TPC, mm_mode: str = MM_MODE, repeat: int = 1,
             skip_mm: bool = False, skip_tr: bool = False, bufs: dict | None = None) -> bass.Bass:
    B = {"x": 3, "xt": 2, "ps_tr": 4, "ps_lg": 2, "rt": 2, "st": 2}
    B.update(bufs or {})
    nt = tokens_per_core // PT
    nc = bacc.Bacc("TRN2", target_bir_lowering=False, debug=False)
    if mm_mode in ("bf16x3p", "bf16x2wp"):
        # host-pretransposed hi/lo: [token, j(hi/lo), kchunk, hidden-in-chunk]
        x_dram = nc.dram_tensor("x", [tokens_per_core, 2, KC, 128], BF16, kind="ExternalInput")
    elif mm_mode == "fp32rp":
        # host-pretransposed fp32: DRAM row (t, p) holds x[t*128 + c, k*128 + p]
        # for all (k, c) — partition p = hidden-in-chunk, free dims [k, c=token]
        x_dram = nc.dram_tensor("x", [tokens_per_core, KC, 128], F32R, kind="ExternalInput")
    else:
        x_dram = nc.dram_tensor("x", [tokens_per_core, HIDDEN], F32, kind="ExternalInput")
    if mm_mode == "fp32":
        w_shape, w_dt, ne = [128, KC, NEXP], F32, NEXP
    elif mm_mode in ("fp32r", "fp32rp"):
        w_shape, w_dt, ne = [128, KC, EPAD], F32R, EPAD
    elif mm_mode in ("bf16x3", "bf16x3p"):
        w_shape, w_dt, ne = [128, KC, 2, NEXP], BF16, NEXP
    elif mm_mode == "bf16x2wp":
        # packed-wide: MM1 streams [Whi|Wlo] (N=320), MM2 streams Whi (N=160)
        w_shape, w_dt, ne = [128, KC, 2, NEXP], BF16, 2 * NEXP
    else:
        raise ValueError(mm_mode)
    # w pre-arranged on host: hidden chunk on partitions (see prep_w)
    w_dram = nc.dram_tensor("w", w_shape, w_dt, kind="ExternalInput")
    out_dram = nc.dram_tensor("out", [tokens_per_core, TOPK], F32, kind="ExternalOutput")

    xt_dt = {"fp32": F32, "fp32r": F32R, "fp32rp": F32R, "bf16x3": BF16,
             "bf16x3p": BF16, "bf16x2wp": BF16}[mm_mode]

    with tile.TileContext(nc) as tc, ExitStack() as ctx:
        const_pool = ctx.enter_context(tc.tile_pool(name="const", bufs=1))
        x_pool = ctx.enter_context(tc.tile_pool(name="x", bufs=B["x"]))
        xt_pool = ctx.enter_context(tc.tile_pool(name="xt", bufs=B["xt"]))
        ps_tr_pool = ctx.enter_context(tc.tile_pool(name="ps_tr", bufs=B["ps_tr"], space="PSUM"))
        ps_lg_pool = ctx.enter_context(tc.tile_pool(name="ps_lg", bufs=B["ps_lg"], space="PSUM"))
        rt_pool = ctx.enter_context(tc.tile_pool(name="rt", bufs=B["rt"]))
        st_pool = ctx.enter_context(tc.tile_pool(name="st", bufs=B["st"]))

        if mm_mode in ("bf16x3p", "bf16x2wp"):
            # split W into per-chunk-group tiles so the first matmuls only
            # wait on the first 0.66MB piece, not the whole 3.3MB load
            WG = 8  # chunks per W group
            w_groups = []
            for g in range(KC // WG):
                wg = const_pool.tile([128, WG, 2, NEXP], w_dt, tag=f"w{g}")
                nc.sync.dma_start(wg[:], w_dram[:, g * WG : (g + 1) * WG])
                w_groups.append(wg)

            def w_ap(k, j):
                return w_groups[k // WG][:, k % WG, j, :]
        else:
            w_sb = const_pool.tile(w_shape, w_dt)
            nc.sync.dma_start(w_sb[:], w_dram[:])
            ident = const_pool.tile([128, 128], F32)
            make_identity(nc, ident[:])

        for t in [i for _ in range(repeat) for i in range(nt)]:
            if mm_mode == "fp32rp":
                xt_t = x_pool.tile([128, KC, 128], F32R, tag="xt")
                nc.sync.dma_start(xt_t[:], x_dram[t * PT : (t + 1) * PT, :, :])
                xt_hi = xt_t[:]
            elif mm_mode in ("bf16x3p", "bf16x2wp"):
                xts = x_pool.tile([128, 2, KC, 128], BF16)
                if t == 0:
                    # ramp: first tile lands hi half in 4 pieces then lo, so
                    # matmuls start after 0.33MB instead of the full 2.6MB
                    for q in range(4):
                        nc.sync.dma_start(xts[:, 0, q * 10 : (q + 1) * 10],
                                          x_dram[0:PT, 0, q * 10 : (q + 1) * 10])
                    nc.sync.dma_start(xts[:, 1], x_dram[0:PT, 1])
                else:
                    nc.sync.dma_start(xts[:], x_dram[t * PT : (t + 1) * PT, :, :, :])
                xt_hi = xts[:, 0]
                xt_lo = xts[:, 1]
            else:
                x_sb = x_pool.tile([128, HIDDEN], F32)
                nc.sync.dma_start(x_sb[:], x_dram[t * PT : (t + 1) * PT, :])

                # transpose phase: x tile -> xT [128 hidden, KC, 128 tokens]
                xt_hi_t = xt_pool.tile([128, KC, 128], xt_dt, tag="xt_hi")
                if mm_mode == "bf16x3":
                    xt_lo_t = xt_pool.tile([128, KC, 128], BF16, tag="xt_lo")
                GK = 4  # transposed chunks per PSUM bank; one batched copy per group
                for g in range(KC // GK):
                    xt_ps = ps_tr_pool.tile([128, GK, 128], F32)
                    if not skip_tr:
                        for j in range(GK):
                            k = g * GK + j
                            nc.tensor.transpose(
                                xt_ps[:, j, :], x_sb[:, k * 128 : (k + 1) * 128], ident[:]
                            )
                    ks = slice(g * GK, (g + 1) * GK)
                    if mm_mode == "bf16x3":
                        # hi = bf16(xT); lo = bf16(xT - hi)
                        nc.scalar.copy(xt_hi_t[:, ks, :], xt_ps[:])
                        nc.vector.tensor_sub(xt_lo_t[:, ks, :], xt_ps[:], xt_hi_t[:, ks, :])
                    else:
                        if g % 2 == 0:
                            nc.vector.tensor_copy(xt_hi_t[:, ks, :], xt_ps[:])
                        else:
                            nc.scalar.copy(xt_hi_t[:, ks, :], xt_ps[:])
                xt_hi = xt_hi_t[:]
                if mm_mode == "bf16x3":
                    xt_lo = xt_lo_t[:]

            # matmul phase: logits[tok, e] += xT_k.T @ W_k
            split3 = mm_mode in ("bf16x3", "bf16x3p")
            lg_ps = ps_lg_pool.tile([128, ne], F32)
            nk = 1 if skip_mm else KC
            if mm_mode == "bf16x3p":
                # interleaved hi,hi,lo per chunk: 2 LDWEIGHTS per 200ns MM
                # stream stay hidden behind the PE background weight buffer.
                # Tile 0 runs the hi terms first so it only waits on the hi
                # DMA half (ramp); steady-state tiles stay interleaved so the
                # lo-term LDWEIGHTS stay hidden.
                if t == 0:
                    for k in range(nk):
                        nc.tensor.matmul(lg_ps[:], xt_hi[:, k, :], w_ap(k, 0),
                                         start=(k == 0), stop=False)
                        nc.tensor.matmul(lg_ps[:], xt_hi[:, k, :], w_ap(k, 1),
                                         start=False, stop=False)
                    for k in range(nk):
                        nc.tensor.matmul(lg_ps[:], xt_lo[:, k, :], w_ap(k, 0),
                                         start=False, stop=(k == nk - 1))
                else:
                    for k in range(nk):
                        nc.tensor.matmul(lg_ps[:], xt_hi[:, k, :], w_ap(k, 0),
                                         start=(k == 0), stop=False)
                        nc.tensor.matmul(lg_ps[:], xt_hi[:, k, :], w_ap(k, 1),
                                         start=False, stop=False)
                        nc.tensor.matmul(lg_ps[:], xt_lo[:, k, :], w_ap(k, 0),
                                         start=False, stop=(k == nk - 1))
            elif mm_mode == "bf16x2wp":
                for k in range(nk):
                    last = k == nk - 1
                    # hi @ [Whi|Wlo] in one N=320 stream; lo @ Whi into cols 0:E
                    wg = w_groups[k // WG]
                    nc.tensor.matmul(
                        lg_ps[:],
                        xt_hi[:, k, :],
                        wg[:, k % WG].rearrange("p j e -> p (j e)"),
                        start=(k == 0), stop=False,
                    )
                    nc.tensor.matmul(lg_ps[:, :NEXP], xt_lo[:, k, :], w_ap(k, 0),
                                     start=False, stop=last)
            else:
                for k in range(nk):
                    last = k == nk - 1
                    if split3:
                        nc.tensor.matmul(lg_ps[:], xt_hi[:, k, :], w_sb[:, k, 0, :],
                                         start=(k == 0), stop=False)
                        nc.tensor.matmul(lg_ps[:], xt_hi[:, k, :], w_sb[:, k, 1, :],
                                         start=False, stop=False)
                        nc.tensor.matmul(lg_ps[:], xt_lo[:, k, :], w_sb[:, k, 0, :],
                                         start=False, stop=last)
                    else:
                        nc.tensor.matmul(lg_ps[:], xt_hi[:, k, :], w_sb[:, k, :],
                                         start=(k == 0), stop=last)

            # routing phase
            if mm_mode == "bf16x2wp":
                logits_sb = st_pool.tile([128, NEXP], F32, tag="logits")
                nc.vector.tensor_tensor(
                    logits_sb[:], lg_ps[:, :NEXP], lg_ps[:, NEXP:],
                    op=mybir.AluOpType.add,
                )
                logits = logits_sb[:]
            else:
                logits = lg_ps[:, :NEXP]
            negmax = rt_pool.tile([128, 1], F32, tag="negmax")
            nc.vector.tensor_reduce(
                negmax[:], logits, axis=mybir.AxisListType.X, op=mybir.AluOpType.max, negate=True
            )
            escore = st_pool.tile([128, NEXP], F32, tag="escore")
            ssum = rt_pool.tile([128, 1], F32, tag="ssum")
            nc.scalar.activation(
                escore[:], logits, mybir.ActivationFunctionType.Exp,
                bias=negmax[:], scale=1.0, accum_out=ssum[:],
            )
            rec = rt_pool.tile([128, 1], F32, tag="rec")
            nc.vector.reciprocal(rec[:], ssum[:])
            # selection is invariant under the positive per-token scale
            # rec*SCALE, so select on raw escore and scale only the top-8
            gs = rt_pool.tile([128, NGROUP], F32, tag="gs")
            nc.vector.tensor_reduce(
                gs[:], escore[:].rearrange("p (g e) -> p g e", e=EPG),
                axis=mybir.AxisListType.X, op=mybir.AluOpType.max,
            )
            g8 = rt_pool.tile([128, 8], F32, tag="g8")
            nc.vector.max(out=g8[:], in_=gs[:])
            gmask = rt_pool.tile([128, NGROUP], F32, tag="gmask")
            nc.vector.tensor_scalar(
                gmask[:], gs[:], g8[:, TOPK_GROUP - 1 : TOPK_GROUP], None,
                op0=mybir.AluOpType.is_ge,
            )
            masked = st_pool.tile([128, NEXP], F32, tag="masked")
            nc.vector.tensor_tensor(
                masked[:].rearrange("p (g e) -> p g e", e=EPG),
                escore[:].rearrange("p (g e) -> p g e", e=EPG),
                gmask[:].to_broadcast([128, NGROUP, EPG]),
                op=mybir.AluOpType.mult,
            )
            top8 = rt_pool.tile([128, 8], F32, tag="top8")
            nc.vector.max(out=top8[:], in_=masked[:])
            scaled8 = rt_pool.tile([128, 8], F32, tag="scaled8")
            nc.vector.tensor_scalar(
                scaled8[:], top8[:], rec[:], SCALE,
                op0=mybir.AluOpType.mult, op1=mybir.AluOpType.mult,
            )
            nc.sync.dma_start(out_dram[t * PT : (t + 1) * PT, :], scaled8[:, :TOPK])

    nc.compile()
    return nc


def _round_fp32r(a: np.ndarray) -> np.ndarray:
    """Round-to-nearest-even to 12-bit significand (tf32-like fp32r)."""
    bits = a.astype(np.float32).view(np.uint32).astype(np.uint64)
    lsb = (bits >> 12) & 1
    rounded = (bits + 0x7FF + lsb) & 0xFFFFF000
    return rounded.astype(np.uint32).view(np.float32)


def prep_w(kernel_w: np.ndarray, mm_mode: str = MM_MODE) -> np.ndarray:
    w = np.asarray(kernel_w, dtype=np.float32)
    if mm_mode == "fp32":
        # [NEXP, HIDDEN] -> [HIDDEN, NEXP] -> [KC, 128, NEXP] -> [128, KC, NEXP]
        return np.ascontiguousarray(w.T.reshape(KC, 128, NEXP).transpose(1, 0, 2))
    if mm_mode in ("fp32r", "fp32rp"):
        wpad = np.zeros((EPAD, HIDDEN), np.float32)
        wpad[:NEXP] = _round_fp32r(w)
        return np.ascontiguousarray(wpad.T.reshape(KC, 128, EPAD).transpose(1, 0, 2))
    if mm_mode in ("bf16x3", "bf16x3p", "bf16x2wp"):
        whi = w.astype(ml_dtypes.bfloat16)
        wlo = (w - whi.astype(np.float32)).astype(ml_dtypes.bfloat16)
        # [2, NEXP, HIDDEN] -> [HIDDEN, 2, NEXP] -> [KC, 128, 2, NEXP] -> [128, KC, 2, NEXP]
        wb = np.stack([whi, wlo])  # [2, NEXP, HIDDEN]
        return np.ascontiguousarray(
            wb.transpose(2, 0, 1).reshape(KC, 128, 2, NEXP).transpose(1, 0, 2, 3)
        )
    raise ValueError(mm_mode)


def prep_x(x: np.ndarray, mm_mode: str = MM_MODE) -> np.ndarray:
    if mm_mode == "fp32rp":
        x = np.asarray(x, dtype=np.float32)
        T = x.shape[0]
        # [t, c, k, p] -> [t, p, k, c]: DRAM row t*128+p (partition = hidden-in-
        # chunk), free dims [k, c] with c = token-in-tile
        X = x.reshape(T // PT, PT, KC, 128).transpose(0, 3, 2, 1)
        return np.ascontiguousarray(X.reshape(T, KC, 128))
    if mm_mode not in ("bf16x3p", "bf16x2wp"):
        return np.ascontiguousarray(x, dtype=np.float32)
    # [t, c, k, p] -> [t, p, j(hi/lo), k, c], tile-blocked so the transpose
    # stays cache-resident (3.2x faster than one global 5-axis gather,
    # bit-identical output)
    x = np.asarray(x, dtype=np.float32)
    T = x.shape[0]
    out = np.empty((T, 2, KC, 128), ml_dtypes.bfloat16)
    xv = x.reshape(T // PT, PT, KC, 128)
    ov = out.reshape(T // PT, PT, 2, KC, 128)
    for t in range(T // PT):
        xt = np.ascontiguousarray(xv[t].transpose(2, 1, 0))  # [128 p, KC, PT]
        hi = xt.astype(ml_dtypes.bfloat16)
        ov[t, :, 0] = hi
        ov[t, :, 1] = (xt - hi.astype(np.float32)).astype(ml_dtypes.bfloat16)
    return out


_NC_CACHE: dict = {}


def run(hidden_states: np.ndarray, kernel_w: np.ndarray, mm_mode: str = MM_MODE, **spmd_kwargs):
    x = prep_x(hidden_states, mm_mode)
    w_arr = prep_w(kernel_w, mm_mode)
    nc = _NC_CACHE.get(mm_mode)
    if nc is None:
        nc = _NC_CACHE[mm_mode] = build_nc(TPC, mm_mode=mm_mode)
    in_maps = [
        {"x": x[i * TPC : (i + 1) * TPC], "w": w_arr} for i in range(NCORES)
    ]
    res = run_bass_kernel_spmd(nc, in_maps, list(range(NCORES)), **spmd_kwargs)
    out = np.concatenate([res.results[i]["out"] for i in range(NCORES)], axis=0)
    return out, res


def kernel(hidden_states: np.ndarray, kernel: np.ndarray) -> np.ndarray:
    return run(hidden_states, kernel)[0]

